# revision 43
# baseline (speedup 1.0000x reference)
"""Trainium2 Bass kernel for nn_MultiHeadAttention_44281112822190.

8 NeuronCores, pure data parallelism over the 8192 (b,s) rows: core c takes
rows [c*1024, (c+1)*1024) (batch b = c//2, s-offset (c%2)*1024). No
collectives; the host shards inputs and reassembles the output.

Math notes:
  - The reference applies RoPE to q and k, then contracts q.k at the SAME
    position (per-position head-head attention [B,S,H,H]). RoPE is an
    orthogonal per-position rotation applied identically to q and k, so it
    cancels exactly in the scores: (R q).(R k) = q.k. The kernel skips RoPE
    entirely (freqs inputs are unused).
  - The reference's "h-major flatten" transpose(0,2,1,3).reshape(B,S,-1) is a
    scramble: out[b, h*128 + s//16, (s%16)*128 + d] = att_out[b, s, h, d].
    Each scrambled row draws from 16 consecutive positions of one head, all
    inside one core's shard, so the output projection stays core-local.

Numerics: all matmul operands are fp16 with fp32 PSUM accumulation ->
~7e-4 relative error end-to-end, 1 cycle/row on the PE.

Perf structure (baseline 584us -> this version):
  - Explicit LDWEIGHTS before each projection matmul: the compile pass
    `move_matmul_waits_to_ldweights` hoists the matmul's semaphore waits
    onto the load and the PE's reorder window hides the load under the
    previous matmul's streaming; with the QKV stream PE-order-chained
    (nosync deps) phase 1 runs at the N=512 streaming roofline (213ns/MM
    vs 259 self-loading).
  - Host pre-tiles weights to [8 t2][128 p][16 k][256 c]: each 1MB weight
    tile DMA is one fully-contiguous 8KB-per-partition read. x is split
    across the scalar+sync HWDGE queues in 8 chunks, first weight tile
    ahead of the consts, so the first matmul starts ~12us earlier.
  - Attention processed in SUPER-UNITS of 4 8-position groups (32
    positions): one exp/recip/normalize/scatter instruction per unit
    instead of two, halving the DVE/ACT/Pool instruction overhead.
  - Each unit is split into stage A (scores+softmax+att-transpose; needs
    only q,k) and stage B (v-transpose+attV+scatter). Stage A of the first
    11 units rides inside the V-projection matmul stream; their stage B
    runs right after; the remaining 21 units pace 1-per-2 output-projection
    half-chunks, so attention stays ~2 units ahead of the out-proj
    consumer and the PE never goes HAM-cold between phases.
  - Out-proj chunks use the same explicit-LDWEIGHTS recipe with
    chunk-local chains only - the Tile scheduler's cost model must stay
    free to slot ready out-proj MMs around attention ops still waiting on
    the softmax chain (a global PE chain measured 240us slower).
  - Output stored fp16 (halves writeback traffic; adds ~2e-4 rel err,
    well inside the 2e-2 gate); out-DMAs on the scalar queue, two feature
    rows per DMA, so weight-tile prefetch owns the sync queue.

Known-bad variants (measured): ldweights=False on InstMatmult is ignored
by codegen (matmuls always self-load); a global PE-order chain through
phase 2 serializes the softmax chain into the PE queue (824us); DMA-XBAR
transposes (dma_start transpose=True) for the att/v slabs race with
something in this pipeline and corrupt the output despite passing a
standalone test.

Measured on trn2 (8 cores, profiled): 522us HW exec, rel err 7.1e-4
(baseline 695us profiled / 584us unprofiled).
"""

import os
import sys

sys.path.insert(0, "/opt/trn_rl_repo")

import numpy as np

import concourse.bacc as bacc
import concourse.mybir as mybir
import concourse.tile as tile
from concourse.bass import _add_dep_helper
from concourse.bass_utils import run_bass_kernel_spmd

F32 = mybir.dt.float32
F16 = mybir.dt.float16
AF = mybir.ActivationFunctionType
ALU = mybir.AluOpType

B, S, E, H, D = 4, 2048, 2048, 16, 128
NCORES = 8
SCALE = 1.0 / float(np.sqrt(D))

_CACHE = {}
LAST_EXEC_NS = None


def _build():
    nc = bacc.Bacc(trn_type="TRN2", target_bir_lowering=False)

    xt = nc.dram_tensor("xt", [128, 16, 1024], F16, kind="ExternalInput")
    wqt = nc.dram_tensor("wqt", [8, 128, 16, 256], F16, kind="ExternalInput")
    wkt = nc.dram_tensor("wkt", [8, 128, 16, 256], F16, kind="ExternalInput")
    wvt = nc.dram_tensor("wvt", [8, 128, 16, 256], F16, kind="ExternalInput")
    wot = nc.dram_tensor("wot", [8, 128, 16, 256], F16, kind="ExternalInput")
    bqt = nc.dram_tensor("bqt", [128, 16], F32, kind="ExternalInput")
    bkt = nc.dram_tensor("bkt", [128, 16], F32, kind="ExternalInput")
    bvt = nc.dram_tensor("bvt", [128, 16], F32, kind="ExternalInput")
    bot = nc.dram_tensor("bot", [128, 16], F32, kind="ExternalInput")
    mask01 = nc.dram_tensor("mask01", [128, 128], F32, kind="ExternalInput")
    ident = nc.dram_tensor("ident", [128, 128], F16, kind="ExternalInput")
    out = nc.dram_tensor("out", [16, 128, 1024], F16, kind="ExternalOutput")

    with tile.TileContext(nc) as tc:
        with (
            tc.tile_pool(name="const", bufs=1) as cp,
            tc.tile_pool(name="xp", bufs=1) as xp,
            tc.tile_pool(name="qkv", bufs=1) as qkvp,
            tc.tile_pool(name="aop", bufs=1) as aop,
            tc.tile_pool(name="wp", bufs=3) as wp,
            tc.tile_pool(name="gp", bufs=4) as gp,
            tc.tile_pool(name="vp2", bufs=2) as vp2,
            tc.tile_pool(name="ep", bufs=1) as ep,
            tc.tile_pool(name="trap", bufs=11) as trap,
            tc.tile_pool(name="op", bufs=3) as op,
            tc.tile_pool(name="pp", bufs=4, space="PSUM") as pp,
            tc.tile_pool(name="pa", bufs=1, space="PSUM") as pa,
            tc.tile_pool(name="pv", bufs=2, space="PSUM") as pv,
            tc.tile_pool(name="pb", bufs=1, space="PSUM") as pb,
        ):
            # Total order on the QKV-projection PE stream = emission order.
            # nosync deps: ordering only, no semaphores.
            last_pe = [None]

            def pe(inst):
                if last_pe[0] is not None:
                    _add_dep_helper(inst.ins, last_pe[0].ins, False, "pe-order")
                last_pe[0] = inst
                return inst

            # first weight tile leads the sync queue; x chunks alternate
            # scalar/sync in consumption order so early matmuls never stall
            xtb_c = [None] * 8
            xc0 = xp.tile([128, 2, 1024], F16, tag="xtb0", name="xtb0")
            nc.scalar.dma_start(xc0[:], xt[:, 0:2, :])
            xtb_c[0] = xc0

            # first weight tile in four k-slices so matmul k=0 starts after
            # 256KB instead of 1MB
            wtile0 = wp.tile([128, 16, 256], F16, tag="w")
            for k4 in range(0, 16, 4):
                nc.sync.dma_start(
                    wtile0[:, k4 : k4 + 4, :], wqt[0, :, k4 : k4 + 4, :]
                )

            for kc in range(1, 8):
                xc = xp.tile([128, 2, 1024], F16, tag=f"xtb{kc}", name=f"xtb{kc}")
                dq = nc.scalar if kc % 2 == 0 else nc.sync
                dq.dma_start(xc[:], xt[:, 2 * kc : 2 * kc + 2, :])
                xtb_c[kc] = xc

            mask_sb = cp.tile([128, 128], F32, tag="mask")
            id_sb = cp.tile([128, 128], F16, tag="id")
            nc.sync.dma_start(mask_sb[:], mask01[:, :])
            nc.sync.dma_start(id_sb[:], ident[:, :])
            bias_sb = {}
            for name, t_ in (("bq", bqt), ("bk", bkt), ("bv", bvt), ("bo", bot)):
                b_sb = cp.tile([128, 16], F32, tag=name)
                nc.sync.dma_start(b_sb[:], t_[:, :])
                bias_sb[name] = b_sb

            # --- Q/K/V projections -> [128 d, 1024 s, 16 h] fp16 ---
            qb = qkvp.tile([128, 1024, 16], F16, tag="qb")
            kb = qkvp.tile([128, 1024, 16], F16, tag="kb")
            vb = qkvp.tile([128, 1024, 16], F16, tag="vb")

            def proj_chunk(wtile, bias, dst, t2, half):
                t = 2 * t2 + half
                psA = pp.tile([128, 512], F32, tag="pp")
                psB = pp.tile([128, 512], F32, tag="pp")
                for k in range(16):
                    w_ap = wtile[:, k, half * 128 : half * 128 + 128]
                    pe(nc.tensor.ldweights(w_ap))
                    pe(nc.tensor.matmul(
                        psA[:], w_ap, xtb_c[k // 2][:, k % 2, 0:512],
                        start=(k == 0), stop=(k == 15),
                    ))
                    pe(nc.tensor.matmul(
                        psB[:], w_ap, xtb_c[k // 2][:, k % 2, 512:1024],
                        start=(k == 0), stop=(k == 15),
                    ))
                nc.vector.tensor_scalar_add(
                    dst[:, 0:512, t], psA[:], bias_sb[bias][:, t : t + 1]
                )
                nc.vector.tensor_scalar_add(
                    dst[:, 512:1024, t], psB[:], bias_sb[bias][:, t : t + 1]
                )

            # --- attention super-units: SP covers groups 4SP..4SP+3
            #     (32 positions), i.e. pairs 2SP and 2SP+1 ---
            # attO half-tiles: [128 d, 16 sl, 256] with col = u_local*16 + h
            attO_h = [
                aop.tile([128, 16, 256], F16, tag=f"attO{q}", name=f"attO{q}")
                for q in range(4)
            ]
            trA_of = {}
            att4_of = {}
            vb_ev_last = [None, None]

            def attn_stageA1(SP):
                # scores + softmax -> normalized att4 (no PE past the scores)
                G = 4 * SP
                gs = pa.tile([128, 512], F32, tag="gs")
                for j in range(4):
                    s0 = (G + j) * 8
                    nc.tensor.matmul(
                        gs[:, 128 * j : 128 * j + 128],
                        qb[:, s0 : s0 + 8, :],
                        kb[:, s0 : s0 + 8, :],
                        start=True, stop=True,
                    )
                e4 = ep.tile([128, 512], F32, tag="e4")
                nc.scalar.activation(e4[:], gs[:], AF.Exp, scale=SCALE)
                em4 = e4[:].rearrange("p (g c) -> p g c", g=4)
                den4 = gp.tile([128, 4], F32, tag="den4")
                for j in range(4):
                    nc.vector.scalar_tensor_tensor(
                        em4[:, j, :], e4[:, 128 * j : 128 * j + 128], 1.0,
                        mask_sb[:], ALU.bypass, ALU.mult,
                        accum_out=den4[:, j : j + 1],
                    )
                rec4 = gp.tile([128, 4], F32, tag="rec4")
                nc.vector.reciprocal(rec4[:], den4[:])
                att4 = gp.tile([128, 4, 128], F16, tag="att4")
                nc.gpsimd.tensor_tensor(
                    att4[:], em4, rec4[:].unsqueeze(2).to_broadcast([128, 4, 128]),
                    ALU.mult,
                )
                att4_of[SP] = att4

            def attn_stageA2(SP):
                # att transposes: emitted ~2 chunks after A1 so the PE never
                # reaches them before the softmax chain has produced att4
                att4 = att4_of.pop(SP)
                trp = pb.tile([128, 512], F16, tag="trp")
                for j in range(4):
                    nc.tensor.transpose(
                        trp[:, 128 * j : 128 * j + 128], att4[:, j, :], id_sb[:]
                    )
                trA = trap.tile([128, 512], F16, tag="trA")
                nc.scalar.activation(trA[:], trp[:], AF.Copy)
                trA_of[SP] = trA

            def attn_stageA(SP):
                attn_stageA1(SP)
                attn_stageA2(SP)

            def attn_stageB(SP):
                G = 4 * SP
                trA = trA_of.pop(SP)
                trp = pb.tile([128, 512], F16, tag="trp")
                for j in range(4):
                    s0 = (G + j) * 8
                    nc.tensor.transpose(
                        trp[:, 128 * j : 128 * j + 128], vb[:, s0 : s0 + 8, :],
                        id_sb[:],
                    )
                trV = vp2.tile([128, 512], F16, tag="trV")
                nc.scalar.activation(trV[:], trp[:], AF.Copy)
                gv = pv.tile([128, 512], F32, tag="gv")
                for j in range(4):
                    nc.tensor.matmul(
                        gv[:, 128 * j : 128 * j + 128],
                        trV[:, 128 * j : 128 * j + 128],
                        trA[:, 128 * j : 128 * j + 128],
                        start=True, stop=True,
                    )
                # scatter both pairs at once:
                # gv col j*128+(i,h) with j=(uo,g2) ->
                #   attO[:, (g2,i), (2*(SP%8)+uo)*16+h]
                dst2 = attO_h[SP // 8][:].rearrange(
                    "p (g2 i) (uo2 uo h) -> p uo2 uo g2 i h", g2=2, uo=2, h=16
                )[:, SP % 8, :, :, :, :]
                nc.vector.tensor_copy(dst2, gv[:])

            # --- phase 1: wq, wk head-major; wv position-half-major with
            # the attention lead units riding inside its matmul stream ---
            NLEAD = 11  # super-units whose stage A rides in the vb stream

            for wi, (wdram, bias, dst) in enumerate(((wqt, "bq", qb), (wkt, "bk", kb))):
                for t2 in range(8):
                    if wi == 0 and t2 == 0:
                        wtile = wtile0
                    else:
                        wtile = wp.tile([128, 16, 256], F16, tag="w")
                        nc.sync.dma_start(wtile[:], wdram[t2, :, :, :])
                    for half in range(2):
                        proj_chunk(wtile, bias, dst, t2, half)

            def vb_chunk(wtile, t2, hp):
                # heads 2t2, 2t2+1 at positions hp*512..hp*512+512
                psA = pp.tile([128, 512], F32, tag="pp")
                psB = pp.tile([128, 512], F32, tag="pp")
                for k in range(16):
                    x_ap = xtb_c[k // 2][:, k % 2, hp * 512 : hp * 512 + 512]
                    wA = wtile[:, k, 0:128]
                    wB = wtile[:, k, 128:256]
                    pe(nc.tensor.ldweights(wA))
                    pe(nc.tensor.matmul(
                        psA[:], wA, x_ap, start=(k == 0), stop=(k == 15)))
                    pe(nc.tensor.ldweights(wB))
                    pe(nc.tensor.matmul(
                        psB[:], wB, x_ap, start=(k == 0), stop=(k == 15)))
                sl = slice(hp * 512, hp * 512 + 512)
                nc.vector.tensor_scalar_add(
                    vb[:, sl, 2 * t2], psA[:], bias_sb["bv"][:, 2 * t2 : 2 * t2 + 1]
                )
                vb_ev_last[hp] = nc.vector.tensor_scalar_add(
                    vb[:, sl, 2 * t2 + 1], psB[:],
                    bias_sb["bv"][:, 2 * t2 + 1 : 2 * t2 + 2],
                )

            # vb pass 0 (positions 0:512): A1 units 0..7, A2 lagged 2 chunks.
            # vb pass 1 (positions 512:1024): A1 8..10, remaining A2s, and
            # stage B of units 0..7 (their v-slabs live in pass-0 positions).
            vb_sched = {
                0: [("A1", 0)], 1: [("A1", 1)], 2: [("A1", 2), ("A2", 0)],
                3: [("A1", 3), ("A2", 1)], 4: [("A1", 4), ("A2", 2)],
                5: [("A1", 5), ("A2", 3)], 6: [("A1", 6), ("A2", 4)],
                7: [("A1", 7), ("A2", 5)],
                8: [("A1", 8), ("A2", 6), ("B", 0)],
                9: [("A1", 9), ("A2", 7), ("B", 1)],
                10: [("A1", 10), ("A2", 8), ("B", 2)],
                11: [("A2", 9), ("B", 3)],
                12: [("A2", 10), ("B", 4)],
                13: [("B", 5), ("B", 6)], 14: [("B", 7)], 15: [],
            }
            c = 0
            for hp in range(2):
                for t2 in range(8):
                    wtile = wp.tile([128, 16, 256], F16, tag="w")
                    nc.sync.dma_start(wtile[:], wvt[t2, :, :, :])
                    vb_chunk(wtile, t2, hp)
                    for kind, s in vb_sched[c]:
                        if kind == "A1":
                            attn_stageA1(s)
                        elif kind == "A2":
                            attn_stageA2(s)
                        else:
                            attn_stageB(s)
                    c += 1

            # --- phase 2 ---
            def final_half(q, t2, half, wtile, ob2):
                # chunk-local LDW/MM chain: keeps loads adjacent to their
                # matmuls without serializing against the attention stream
                t = 2 * t2 + half
                ps = pp.tile([128, 512], F32, tag="pp")
                prev = None
                for sl in range(16):
                    w_ap = wtile[:, sl, half * 128 : half * 128 + 128]
                    ldw = nc.tensor.ldweights(w_ap)
                    if prev is not None:
                        _add_dep_helper(ldw.ins, prev.ins, False, "chunk-order")
                    m = nc.tensor.matmul(
                        ps[:, 0:256], w_ap, attO_h[q][:, sl, :],
                        start=(sl == 0), stop=(sl == 15),
                    )
                    _add_dep_helper(m.ins, ldw.ins, False, "chunk-order")
                    prev = m
                # evictions alternate Vector/Scalar so neither queue's
                # backlog delays freeing the out-proj PSUM banks
                if half == 0:
                    nc.vector.tensor_scalar_add(
                        ob2[:, half, :], ps[:, 0:256], bias_sb["bo"][:, t : t + 1]
                    )
                else:
                    nc.scalar.activation(
                        ob2[:, half, :], ps[:, 0:256], AF.Identity,
                        bias=bias_sb["bo"][:, t : t + 1],
                    )

            # remaining work: stage B of lead units 8..10, then full units
            # 11..31, software-pipelined so unit s's att-transposes (A2) are
            # one pacing slot (~4us) behind its softmax chain (A1) - with
            # zero lag the in-order PE stalls ~2us per unit on the GpSimd
            # normalize. Four slots front-loaded, then one per two halves.
            sched = [[("B", s)] for s in range(8, NLEAD)]
            sched.append([("A1", NLEAD)])
            for s in range(NLEAD + 1, 32):
                sched.append([("A2B", s - 1), ("A1", s)])
            sched.append([("A2B", 31)])
            widx = 0
            h = 0
            for q in range(4):
                for t2 in range(8):
                    wtile = wp.tile([128, 16, 256], F16, tag="w")
                    nc.sync.dma_start(wtile[:], wot[t2, :, :, :])
                    ob2 = op.tile([128, 2, 256], F16, tag="ob")
                    for half in range(2):
                        final_half(q, t2, half, wtile, ob2)
                        if (h < 4 or h % 2 == 0) and widx < len(sched):
                            for kind, s in sched[widx]:
                                if kind == "B":
                                    attn_stageB(s)
                                elif kind == "A1":
                                    attn_stageA1(s)
                                else:
                                    attn_stageA2(s)
                                    attn_stageB(s)
                            widx += 1
                        h += 1
                    # last quarter: sync queue is idle once its weight tiles
                    # are in; scalar's DMA ring credits stall its evictions
                    dq = nc.sync if q == 3 else nc.scalar
                    dq.dma_start(
                        out[2 * t2 : 2 * t2 + 2, :, q * 256 : q * 256 + 256]
                        .rearrange("t p c -> p t c"),
                        ob2[:],
                    )

    nc.compile()
    return nc


def _get_nc():
    if "nc" not in _CACHE:
        _CACHE["nc"] = _build()
    return _CACHE["nc"]


def make_in_maps(inputs):
    x = np.ascontiguousarray(np.asarray(inputs["x"], dtype=np.float32))
    ws = {k: np.asarray(inputs[k], dtype=np.float32) for k in ("wq", "wk", "wv", "wo")}
    bs = {k: np.asarray(inputs[k], dtype=np.float32) for k in ("bq", "bk", "bv", "bo")}

    xf = x.reshape(B * S, E)
    f16 = lambda a: np.ascontiguousarray(a).astype(np.float16)
    btile = lambda b: np.ascontiguousarray(b.reshape(16, 128).T)
    # weight tiles laid out [8 t2][128 p][16 k][256 c]: row e = k*128+p,
    # col = t2*256+c of w.T, so each tile DMA is contiguous per partition.
    wtile4 = lambda wT: np.ascontiguousarray(
        f16(wT).reshape(16, 128, 8, 256).transpose(2, 1, 0, 3)
    )
    ii = np.arange(128) // 16
    mask01 = (ii[:, None] == ii[None, :]).astype(np.float32)
    common = {
        "wqt": wtile4(ws["wq"].T), "wkt": wtile4(ws["wk"].T),
        "wvt": wtile4(ws["wv"].T), "wot": wtile4(ws["wo"].T),
        "bqt": btile(bs["bq"]), "bkt": btile(bs["bk"]),
        "bvt": btile(bs["bv"]), "bot": btile(bs["bo"]),
        "mask01": mask01, "ident": np.eye(128, dtype=np.float16),
    }
    in_maps = []
    for c in range(NCORES):
        # x chunk as [128 p][16 k][1024 s] (fully contiguous per partition)
        xt_c = np.ascontiguousarray(
            f16(xf[c * 1024 : (c + 1) * 1024].T).reshape(16, 128, 1024)
            .transpose(1, 0, 2)
        )
        in_maps.append({"xt": xt_c, **common})
    return in_maps


def assemble(results):
    out = np.empty((B, S, E), np.float32)
    for c in range(NCORES):
        O = results[c]["out"].astype(np.float32)  # [16 t, 128 p, 1024]; col = u*16+h
        Oc = O.reshape(E, 64, 16)  # [j, u, h]
        tgt = out[c // 2].reshape(16, 128, E)
        v0 = (c % 2) * 64
        tgt[:, v0 : v0 + 64, :] = Oc.transpose(2, 1, 0)
    return out


def kernel(**inputs):
    global LAST_EXEC_NS
    nc = _get_nc()
    res = run_bass_kernel_spmd(nc, make_in_maps(inputs), core_ids=list(range(NCORES)))
    LAST_EXEC_NS = res.exec_time_ns
    return assemble(res.results)
